# revision 8
# baseline (speedup 1.0000x reference)
"""LIF layer kernel for Trainium2 (8 NeuronCores, SPMD batch-parallel).

Problem: x [T=32, B=128, N=8192] f32. Recurrence per (b, n):
    v    = 0.5*v + x[t]
    s    = (v - 0.5 > 0)
    v    = v * (1 - s)
    o[t] = s * 0.5

Sharding: batch is split 8 ways (16 batches/core). Per core the per-step
slab of 16*8192 = 131072 elements is viewed as an SBUF tile [128, 1024].
The time recurrence is elementwise, so no cross-core communication.

Per-step compute (bit-exact vs the fp32 reference):
    u = A + x_t                  DVE tensor_tensor add
    m = (u <= 0.5) * 0.5         DVE tensor_scalar (fused two-op)
    o = 0.5 - m                  ScalarE Copy(scale=-1, bias=0.5) [spike out]
    A = u * m                    DVE tensor_tensor mult [= 0.5 * v_reset]
with A0 = 0. (scalar_tensor_tensor would fuse the scale into the add but
trips a walrus codegen limit: "Too many sync wait commands" on S2S2D2_STT.)

DMA: timesteps are grouped into chunks of G=8 (4 MiB per transfer per
direction) to amortize the ~2us fixed dma_start cost; HWDGE (nc.sync)
so descriptor generation never contends with DVE perf-mode ops.
"""

import numpy as np

import concourse.bass as bass
import concourse.mybir as mybir
from concourse.bass_utils import run_bass_kernel_spmd
from concourse.tile import TileContext

FP32 = mybir.dt.float32

T = 32          # timesteps
B = 128         # global batch
N = 8192        # features
N_CORES = 8
B_SH = B // N_CORES          # 16 batches per core
FD = B_SH * N // 128         # 1024 free-dim per partition per step
G = 8                        # timesteps per DMA chunk

THRESH = 0.5
DECAY = 0.5


def build_lif(T=T, FD=FD, G=G, mask_engine="dve", legalize=True):
    """Build the per-core Bass program. DRAM I/O: x [T,128,FD] -> o [T,128,FD].

    mask_engine: which engine computes m = (u <= 0.5)*0.5 ("dve" or "gpsimd").
    """
    nc = bass.Bass(trn_type="TRN2")
    x = nc.declare_dram_parameter("x", [T, 128, FD], FP32, isOutput=False)
    o = nc.declare_dram_parameter("o", [T, 128, FD], FP32, isOutput=True)

    n_chunks = T // G
    with TileContext(nc) as tc:
        with (
            tc.tile_pool(name="io", bufs=2) as io_pool,
            tc.tile_pool(name="state", bufs=2) as state_pool,
        ):
            # A = DECAY * (post-reset membrane); u_t = A + x_t
            a = state_pool.tile([128, FD], FP32, tag="a")
            nc.vector.memset(a, 0.0)

            mask_eng = nc.gpsimd if mask_engine == "gpsimd" else nc.vector

            for c in range(n_chunks):
                xin = io_pool.tile([128, G, FD], FP32, tag="xin")
                yout = io_pool.tile([128, G, FD], FP32, tag="yout")
                nc.sync.dma_start(
                    out=xin, in_=x[c * G:(c + 1) * G].rearrange("g p f -> p g f")
                )
                for s in range(G):
                    xt = xin[:, s, :]
                    u = state_pool.tile([128, FD], FP32, tag="u")
                    # u = A + x_t
                    nc.vector.tensor_tensor(
                        out=u, in0=a, in1=xt, op=mybir.AluOpType.add,
                    )
                    # m = (u <= THRESH) * DECAY   (in {0, 0.5})
                    m = state_pool.tile([128, FD], FP32, tag="m")
                    mask_eng.tensor_scalar(
                        out=m, in0=u, scalar1=THRESH, scalar2=DECAY,
                        op0=mybir.AluOpType.is_le, op1=mybir.AluOpType.mult,
                    )
                    # o_t = 0.5 - m  (spike * THRESH), off the critical path
                    nc.scalar.activation(
                        out=yout[:, s, :], in_=m,
                        func=mybir.ActivationFunctionType.Copy,
                        bias=THRESH, scale=-1.0,
                    )
                    # A = u * m   (hard reset where spiked, pre-scaled by DECAY)
                    a = state_pool.tile([128, FD], FP32, tag="a")
                    nc.vector.tensor_tensor(
                        out=a, in0=u, in1=m, op=mybir.AluOpType.mult,
                    )
                nc.sync.dma_start(
                    out=o[c * G:(c + 1) * G].rearrange("g p f -> p g f"), in_=yout
                )
    return _legalize_sync_waits(nc) if legalize else nc


def _legalize_sync_waits(nc):
    """Split multi-sem waits: this walrus codegen allows only one sync-wait
    per compute instruction ("Too many sync wait commands"). Hoist all but
    one wait onto InstNoOp carriers inserted just before, on the same engine
    (same program position, so scheduling semantics are unchanged)."""
    k = 0
    for fn in nc.m.functions:
        for bb in fn.blocks:
            out = []
            for inst in bb.instructions:
                si = getattr(inst, "sync_info", None)
                if si is not None and len(si.on_wait) > 1:
                    waits = list(si.on_wait)
                    for w in waits[:-1]:
                        nop = mybir.InstNoOp(name=f"legal-wait-{k}")
                        k += 1
                        nop.engine = inst.engine
                        nop.sync_info = mybir.SyncInfo(on_wait=[w], on_update=[])
                        out.append(nop)
                    si.on_wait = waits[-1:]
                    inst.sync_info = si
                out.append(inst)
            bb.instructions = out
    return nc


_NC_CACHE = {}


def _get_nc():
    if "nc" not in _NC_CACHE:
        _NC_CACHE["nc"] = build_lif()
    return _NC_CACHE["nc"]


def kernel(x: np.ndarray, _trace: bool = False, _result_holder: dict | None = None):
    """Full-input entry point: x [32, 128, 8192] f32 -> spikes [32, 128, 8192] f32."""
    assert x.shape == (T, B, N) and x.dtype == np.float32
    in_maps = []
    for c in range(N_CORES):
        shard = np.ascontiguousarray(x[:, c * B_SH:(c + 1) * B_SH, :])
        in_maps.append({"x": shard.reshape(T, 128, FD)})

    nc = _get_nc()
    res = run_bass_kernel_spmd(nc, in_maps, core_ids=list(range(N_CORES)),
                               trace=_trace)
    if _result_holder is not None:
        _result_holder["res"] = res

    out = np.empty((T, B, N), dtype=np.float32)
    for c in range(N_CORES):
        out[:, c * B_SH:(c + 1) * B_SH, :] = (
            res.results[c]["o"].reshape(T, B_SH, N)
        )
    return out
